# revision 7
# baseline (speedup 1.0000x reference)
"""Trainium2 Bass kernel for nn_Attention (dense transformer attention layer).

Reference semantics (bug-faithful to the source):
  - Q = x @ wq.T ; V = x @ wv.T ; K-projection is DEAD CODE (the reference
    overwrites xk with the double-angle-rotated Q, so wk never matters).
  - rot = double-angle RoPE applied to Q; keys == rot(Q).
  - start_pos == 0 and t == MAX_SEQ, so the KV cache contents never matter.
  - scores = rotQ @ rotQ.T / sqrt(HD) + mask ; P = softmax ; O = P @ V
  - out = O @ wo.T

Sharding (8 cores): core c -> batch b = c//2, head-half h = c%2 (8 of 16
heads).  Q/V projections + attention are (batch x head-half) parallel; the
attention outputs are exchanged pairwise (AllToAll over [2b, 2b+1]) so the
output projection runs (batch x token-half) parallel with a full-D
contraction and no reduction.

Layout strategy: all matmul contractions need the contracted dim on SBUF
partitions, so the host pre-transposes x / wq / wv / wo / mask (pure layout
prep in kernel()).  Compute dtype is bf16 on the TensorEngine (fp32 PSUM
accumulation); softmax runs in fp32 without max-subtraction (scores*scale
<= ~15, exp is safe).  The score matrix is symmetric (keys == queries), so
scores are produced directly in [k, q] layout and P^T feeds the PV matmul
with no transposes; softmax denominators come from a ones-vector matmul
accumulated alongside PV.
"""

import math
import sys

import numpy as np

sys.path.insert(0, "/opt/trn_rl_repo")

import concourse.bacc as bacc
import concourse.mybir as mybir
from concourse.tile import TileContext

F32 = mybir.dt.float32
BF16 = mybir.dt.bfloat16

B = 4
T = 2048
D = 2048
H = 16
HD = 128
N_CORES = 8
PAIRS = [[0, 1], [2, 3], [4, 5], [6, 7]]


def mask_blocks(T):
    """(k_tile, q_tile) blocks of mask^T that must be applied: the diagonal
    blocks plus the in-chunk wedge (k_tile > q_tile) blocks."""
    NCH = max(1, T // 512)
    QPC = (T // NCH) // 128  # q-tiles per chunk
    blocks = []
    for c in range(NCH):
        for qt in range(c * QPC, (c + 1) * QPC):
            for kt in range(qt, (c + 1) * QPC):
                blocks.append((kt, qt))
    return blocks


def build_nc(T, D, H):
    HD = 128
    assert D == H * HD
    NH = H // 2          # heads per core
    DQ = NH * HD         # own q/v feature count
    TH = T // 2          # token half
    NT = T // 128        # token tiles
    ND = D // 128        # d tiles
    NCH = max(1, T // 512)
    CH = T // NCH        # q-chunk width (512)
    QPC = CH // 128      # q-tiles per chunk
    NQ4 = T // 512 if T >= 512 else 1   # xt load quarters
    XTQ = T // NQ4       # tokens per xt load chunk
    G = 2                # a2a groups
    HG = max(1, NH // G) # heads per group
    DQC = max(1, DQ // 512)   # dq chunks of <=512
    DQW = DQ // DQC           # dq chunk width
    HPC = DQW // HD           # heads per dq chunk
    blocks = mask_blocks(T)
    NBLK = len(blocks)
    blkidx = {b: i for i, b in enumerate(blocks)}
    scale = 1.0 / math.sqrt(HD)

    nc = bacc.Bacc(target_bir_lowering=False, num_devices=N_CORES)

    xt = nc.declare_dram_parameter("xt", [D, T], F32, isOutput=False)
    wqt = nc.declare_dram_parameter("wqt", [D, DQ], F32, isOutput=False)
    wvt = nc.declare_dram_parameter("wvt", [D, DQ], F32, isOutput=False)
    wot = nc.declare_dram_parameter("wot", [D, D], F32, isOutput=False)
    mkt = nc.declare_dram_parameter("maskt", [128, NBLK * 128], F32, isOutput=False)
    fc = nc.declare_dram_parameter("fc", [128, NT * 64], F32, isOutput=False)
    fs = nc.declare_dram_parameter("fs", [128, NT * 64], F32, isOutput=False)
    ident = nc.declare_dram_parameter("ident", [128, 128], F32, isOutput=False)
    out = nc.declare_dram_parameter("out", [TH, D], F32, isOutput=True)

    # pair exchange: each core AllGathers the token-half it gives away;
    # ag_out[i] = rank i's contribution (i = index within the pair)
    ag_in = [nc.dram_tensor(f"agi{g}", [HG * 128, TH], BF16) for g in range(G)]
    ag_out = [nc.dram_tensor(f"ago{g}", [2, HG * 128, TH], BF16) for g in range(G)]

    with TileContext(nc) as tc:
        import concourse.bass as bass_mod

        pid = nc.partition_id()
        h_idx = pid % 2
        off_own = h_idx * TH        # this core's token-half offset
        off_peer = (1 - h_idx) * TH  # the half it gives away
        peer_i = 1 - h_idx           # peer's index within the pair
        with (
            tc.tile_pool(name="persist", bufs=1) as pp,
            tc.tile_pool(name="rotqt", bufs=1) as rqp,
            tc.tile_pool(name="vsb", bufs=1) as vp,
        ):
            # --- persistent tiles ---
            rotqt = rqp.tile([128, NH * T], BF16, tag="rotqt")
            v_sb = vp.tile([128, NT * DQ], BF16, tag="vsb")
            ones_sb = pp.tile([128, 1], BF16, tag="ones")
            ident_sb = pp.tile([128, 128], F32, tag="ident")
            c2 = pp.tile([128, NT * 64], F32, tag="c2")
            s2 = pp.tile([128, NT * 64], F32, tag="s2")
            fc_sb = pp.tile([128, NT * 64], F32, tag="fcs")
            fs_sb = pp.tile([128, NT * 64], F32, tag="fss")

            nc.vector.memset(ones_sb[:, :], 1.0)
            nc.sync.dma_start(out=ident_sb[:, :], in_=ident[:, :])
            nc.sync.dma_start(out=fc_sb[:, :], in_=fc[:, :])
            nc.sync.dma_start(out=fs_sb[:, :], in_=fs[:, :])
            # double-angle tables: c2 = fc^2 - fs^2 ; s2 = 2 fc fs
            nc.vector.tensor_mul(c2[:, :], fc_sb[:, :], fc_sb[:, :])
            nc.vector.tensor_mul(s2[:, :], fs_sb[:, :], fs_sb[:, :])
            nc.vector.tensor_sub(c2[:, :], c2[:, :], s2[:, :])
            nc.vector.tensor_mul(s2[:, :], fc_sb[:, :], fs_sb[:, :])
            nc.vector.tensor_scalar_mul(s2[:, :], s2[:, :], 2.0)

            # ---------------- phase 1: projections + rope -----------------
            with (
                tc.tile_pool(name="p1sbuf", bufs=2) as p1,
                tc.tile_pool(name="p1w", bufs=1) as p1w,
                tc.tile_pool(name="p1tmp", bufs=3) as p1t,
                tc.tile_pool(name="p1ps", bufs=2, space="PSUM") as p1ps,
                tc.tile_pool(name="p1pst", bufs=2, space="PSUM") as p1pst,
            ):
                wqt_sb = p1w.tile([128, ND * DQ], BF16, tag="wqt")
                wvt_sb = p1w.tile([128, ND * DQ], BF16, tag="wvt")
                for dk in range(ND):
                    nc.gpsimd.dma_start(
                        out=wqt_sb[:, dk * DQ : (dk + 1) * DQ],
                        in_=wqt[dk * 128 : (dk + 1) * 128, :],
                    )
                for dk in range(ND):
                    nc.gpsimd.dma_start(
                        out=wvt_sb[:, dk * DQ : (dk + 1) * DQ],
                        in_=wvt[dk * 128 : (dk + 1) * 128, :],
                    )

                for tq in range(NQ4):
                    xt_sb = p1.tile([128, ND * XTQ], BF16, tag="xt")
                    for dk in range(ND):
                        nc.gpsimd.dma_start(
                            out=xt_sb[:, dk * XTQ : (dk + 1) * XTQ],
                            in_=xt[dk * 128 : (dk + 1) * 128, tq * XTQ : (tq + 1) * XTQ],
                        )
                    for tt in range(XTQ // 128):
                        tb = tq * (XTQ // 128) + tt  # global token tile

                        def xt_tile(dk):
                            return xt_sb[:, dk * XTQ + tt * 128 : dk * XTQ + tt * 128 + 128]

                        # --- Q projection + rope + transpose ---
                        for qc in range(DQC):
                            ps_q = p1ps.tile([128, DQW], F32, tag="psq")
                            for dk in range(ND):
                                nc.tensor.matmul(
                                    ps_q[:, :],
                                    xt_tile(dk),
                                    wqt_sb[:, dk * DQ + qc * DQW : dk * DQ + (qc + 1) * DQW],
                                    start=(dk == 0),
                                    stop=(dk == ND - 1),
                                )
                            # rope (double angle), writes deinterleaved r|i per head
                            nh = HPC
                            qr = ps_q[:, 0:DQW:2].rearrange("p (h i) -> p h i", h=nh)
                            qi = ps_q[:, 1:DQW:2].rearrange("p (h i) -> p h i", h=nh)
                            c2b = (
                                c2[:, tb * 64 : tb * 64 + 64]
                                .unsqueeze(1)
                                .broadcast_to((128, nh, 64))
                            )
                            s2b = (
                                s2[:, tb * 64 : tb * 64 + 64]
                                .unsqueeze(1)
                                .broadcast_to((128, nh, 64))
                            )
                            rq = p1t.tile([128, DQW], F32, tag="rotqnat")
                            rqv = rq[:, :].rearrange("p (h two i) -> p h two i", two=2, i=64)
                            t1 = p1t.tile([128, nh * 64], F32, tag="t1")
                            t2 = p1t.tile([128, nh * 64], F32, tag="t2")
                            t1v = t1[:, :].rearrange("p (h i) -> p h i", h=nh)
                            t2v = t2[:, :].rearrange("p (h i) -> p h i", h=nh)
                            nc.vector.tensor_mul(t1v, qr, c2b)
                            nc.vector.tensor_mul(t2v, qi, s2b)
                            nc.vector.tensor_sub(rqv[:, :, 0, :], t1v, t2v)
                            nc.vector.tensor_mul(t1v, qr, s2b)
                            nc.vector.tensor_mul(t2v, qi, c2b)
                            nc.vector.tensor_add(rqv[:, :, 1, :], t1v, t2v)
                            # transpose each head block -> rotqt
                            for hl in range(nh):
                                eta = qc * HPC + hl
                                ps_t = p1pst.tile([128, 128], F32, tag="pst")
                                nc.tensor.transpose(
                                    ps_t[:, :],
                                    rq[:, hl * 128 : (hl + 1) * 128],
                                    ident_sb[:, :],
                                )
                                nc.vector.tensor_copy(
                                    rotqt[:, eta * T + tb * 128 : eta * T + tb * 128 + 128],
                                    ps_t[:, :],
                                )

                        # --- V projection ---
                        for qc in range(DQC):
                            ps_v = p1ps.tile([128, DQW], F32, tag="psv")
                            for dk in range(ND):
                                nc.tensor.matmul(
                                    ps_v[:, :],
                                    xt_tile(dk),
                                    wvt_sb[:, dk * DQ + qc * DQW : dk * DQ + (qc + 1) * DQW],
                                    start=(dk == 0),
                                    stop=(dk == ND - 1),
                                )
                            nc.vector.tensor_copy(
                                v_sb[:, tb * DQ + qc * DQW : tb * DQ + (qc + 1) * DQW],
                                ps_v[:, :],
                            )

            # ---------------- phase 2: attention + a2a + out proj ----------
            with (
                tc.tile_pool(name="p2sbuf", bufs=1) as p2,
                tc.tile_pool(name="p2pt", bufs=4) as ptp,
                tc.tile_pool(name="p2ot", bufs=2) as otp,
                tc.tile_pool(name="p2wot", bufs=2) as wotp,
                tc.tile_pool(name="p2of", bufs=1) as ofp,
                tc.tile_pool(name="p2outsb", bufs=3) as outp,
                tc.tile_pool(name="p2rc", bufs=2) as rcp,
                tc.tile_pool(name="psS", bufs=2, space="PSUM") as psS,
                tc.tile_pool(name="psO", bufs=2, space="PSUM") as psO,
                tc.tile_pool(name="psD", bufs=2, space="PSUM") as psD,
                tc.tile_pool(name="psOut", bufs=2, space="PSUM") as psOut,
            ):
                mkt_sb = p2.tile([128, NBLK * 128], BF16, tag="mkt")
                nc.gpsimd.dma_start(out=mkt_sb[:, :], in_=mkt[:, :])
                o_full = ofp.tile([128, 2 * NH * TH], BF16, tag="ofull")

                for eta in range(NH):
                    ot_sb = otp.tile([128, T], BF16, tag="ot")
                    for c in range(NCH):
                        q0 = c * CH
                        KC = (c + 1) * QPC  # k tiles for this chunk
                        ps_o = psO.tile([128, CH], F32, tag="pso")
                        ps_d = psD.tile([1, CH], F32, tag="psd")
                        for kt in range(KC):
                            ps_s = psS.tile([128, CH], F32, tag="pss")
                            nc.tensor.matmul(
                                ps_s[:, :],
                                rotqt[:, eta * T + kt * 128 : eta * T + kt * 128 + 128],
                                rotqt[:, eta * T + q0 : eta * T + q0 + CH],
                                start=True,
                                stop=True,
                            )
                            for qt in range(c * QPC, min(kt, (c + 1) * QPC - 1) + 1):
                                bi = blkidx[(kt, qt)]
                                qo = (qt - c * QPC) * 128
                                nc.vector.tensor_add(
                                    ps_s[:, qo : qo + 128],
                                    ps_s[:, qo : qo + 128],
                                    mkt_sb[:, bi * 128 : bi * 128 + 128],
                                )
                            pt = ptp.tile([128, CH], BF16, tag="pt")
                            nc.scalar.activation(
                                pt[:, :],
                                ps_s[:, :],
                                mybir.ActivationFunctionType.Exp,
                                scale=scale,
                            )
                            nc.tensor.matmul(
                                ps_o[:, :],
                                v_sb[:, kt * DQ + eta * 128 : kt * DQ + eta * 128 + 128],
                                pt[:, :],
                                start=(kt == 0),
                                stop=(kt == KC - 1),
                            )
                            nc.tensor.matmul(
                                ps_d[:, :],
                                ones_sb[:, :],
                                pt[:, :],
                                start=(kt == 0),
                                stop=(kt == KC - 1),
                            )
                        rcpv = rcp.tile([1, CH], F32, tag="rcp")
                        rcpb = rcp.tile([128, CH], F32, tag="rcpb")
                        nc.vector.reciprocal(rcpv[:, :], ps_d[:, :])
                        nc.gpsimd.partition_broadcast(rcpb[:, :], rcpv[:, :])
                        nc.vector.tensor_mul(
                            ot_sb[:, q0 : q0 + CH], ps_o[:, :], rcpb[:, :]
                        )
                    g = eta // HG
                    hg = eta % HG
                    # keep own token-half locally (o_full rows are in LOCAL
                    # head order: 0..NH-1 own heads, NH..2NH-1 peer heads;
                    # the host permutes wot rows per core to match)
                    nc.vector.tensor_copy(
                        o_full[:, eta * TH : (eta + 1) * TH],
                        ot_sb[:, bass_mod.ds(off_own, TH)],
                    )
                    nc.gpsimd.dma_start(
                        out=ag_in[g][hg * 128 : (hg + 1) * 128, :],
                        in_=ot_sb[:, bass_mod.ds(off_peer, TH)],
                    )
                    if eta == (g + 1) * HG - 1:
                        nc.gpsimd.collective_compute(
                            "AllGather",
                            mybir.AluOpType.bypass,
                            replica_groups=PAIRS,
                            ins=[ag_in[g].ap().opt()],
                            outs=[ag_out[g].ap().opt()],
                        )
                        # peer's give-away half -> o_full peer rows
                        for hg2 in range(HG):
                            d2t = NH + g * HG + hg2
                            nc.gpsimd.dma_start(
                                out=o_full[:, d2t * TH : (d2t + 1) * TH],
                                in_=ag_out[g][
                                    bass_mod.ds(peer_i, 1),
                                    hg2 * 128 : (hg2 + 1) * 128,
                                    :,
                                ],
                            )

                # ---- output projection (token-half sharded, full D) ----
                d2order = list(range(2 * NH))
                NDO = max(1, D // 512)
                DOW = D // NDO
                for do in range(NDO):
                    wot_sb = wotp.tile([128, ND * DOW], BF16, tag="wot")
                    for dk in range(ND):
                        nc.gpsimd.dma_start(
                            out=wot_sb[:, dk * DOW : (dk + 1) * DOW],
                            in_=wot[dk * 128 : (dk + 1) * 128, do * DOW : (do + 1) * DOW],
                        )
                    for tb8 in range(TH // 128):
                        ps_out = psOut.tile([128, DOW], F32, tag="psout")
                        for n, d2t in enumerate(d2order):
                            nc.tensor.matmul(
                                ps_out[:, :],
                                o_full[:, d2t * TH + tb8 * 128 : d2t * TH + tb8 * 128 + 128],
                                wot_sb[:, d2t * DOW : (d2t + 1) * DOW],
                                start=(n == 0),
                                stop=(n == len(d2order) - 1),
                            )
                        osb = outp.tile([128, DOW], F32, tag="osb")
                        nc.vector.tensor_copy(osb[:, :], ps_out[:, :])
                        nc.sync.dma_start(
                            out=out[tb8 * 128 : (tb8 + 1) * 128, do * DOW : (do + 1) * DOW],
                            in_=osb[:, :],
                        )

    nc.finalize()
    return nc


def host_prep(T, D, H, x, wq, wv, wo, mask, freqs_cos, freqs_sin):
    """Build per-core in_maps (host-side layout prep only)."""
    HD = 128
    NH = H // 2
    NT = T // 128
    blocks = mask_blocks(T)
    m = np.asarray(mask, np.float32).reshape(T, T)
    mkt = np.concatenate(
        [m[qt * 128 : (qt + 1) * 128, kt * 128 : (kt + 1) * 128].T for (kt, qt) in blocks],
        axis=1,
    )
    fcn = np.asarray(freqs_cos, np.float32)
    fsn = np.asarray(freqs_sin, np.float32)
    fc = np.ascontiguousarray(
        fcn.reshape(NT, 128, 64).transpose(1, 0, 2).reshape(128, NT * 64)
    )
    fsh = np.ascontiguousarray(
        fsn.reshape(NT, 128, 64).transpose(1, 0, 2).reshape(128, NT * 64)
    )
    ident = np.eye(128, dtype=np.float32)
    wot = np.ascontiguousarray(np.asarray(wo, np.float32).T)  # [din2, dout]
    DQ = NH * HD
    in_maps = []
    for c in range(N_CORES):
        b, h = c // 2, c % 2
        rows = slice(h * NH * HD, (h + 1) * NH * HD)
        # o_full rows are in local head order (own heads first), so permute
        # wot's din2 rows to match
        wot_c = np.ascontiguousarray(
            np.concatenate(
                [wot[h * DQ : (h + 1) * DQ], wot[(1 - h) * DQ : (2 - h) * DQ]], axis=0
            )
        )
        in_maps.append(
            {
                "xt": np.ascontiguousarray(np.asarray(x[b], np.float32).T),
                "wqt": np.ascontiguousarray(np.asarray(wq[rows], np.float32).T),
                "wvt": np.ascontiguousarray(np.asarray(wv[rows], np.float32).T),
                "wot": wot_c,
                "maskt": np.ascontiguousarray(mkt),
                "fc": fc,
                "fs": fsh,
                "ident": ident,
            }
        )
    return in_maps


_NC_CACHE = {}


def run(T, D, H, inputs, trace=False):
    from concourse.bass_utils import run_bass_kernel_spmd

    key = (T, D, H)
    if key not in _NC_CACHE:
        _NC_CACHE[key] = build_nc(T, D, H)
    nc = _NC_CACHE[key]
    in_maps = host_prep(
        T, D, H,
        inputs["x"], inputs["wq"], inputs["wv"], inputs["wo"],
        inputs["mask"], inputs["freqs_cos"], inputs["freqs_sin"],
    )
    res = run_bass_kernel_spmd(nc, in_maps, core_ids=list(range(N_CORES)), trace=trace)
    B_ = np.asarray(inputs["x"]).shape[0]
    TH = T // 2
    out = np.empty((B_, T, D), np.float32)
    for c in range(N_CORES):
        b, h = c // 2, c % 2
        out[b, h * TH : (h + 1) * TH, :] = res.results[c]["out"]
    return out, res


def kernel(**inputs):
    out, _ = run(T, D, H, inputs, trace=False)
    return out
